# revision 67
# baseline (speedup 1.0000x reference)
"""MultiHeadEMA (MEGA bidirectional EMA + residual + SiLU) on 8 Trainium2 cores.

Strategy
--------
Per channel d (E=1024, B=4, L=4096):
    y[n] = silu( sum_{m<=n} x[m] k1[d, n-m] + sum_{m>n} x[m] k2[d, m-n-1]
                 + omega[d] x[n] )
with k1/k2 16-term geometric mixtures, q = 1 - sigmoid(a)*sigmoid(d) <= 0.865.
q^32 tail: worst-channel L1 1.4e-2 << 2e-2 * scale(16.9), so the length-2L FFT
conv reduces to a +-T=32-tap banded conv done by overlap-save with DFT F=256,
hop C=192 (22 windows).  E sharded 8 ways (128 ch/core, FREE = B*128 = 512).

Matmul cost on TRN2 is free-dim cycles (~216 ns at N=512 fp16), so every
matmul is full 128x128 config.  x is staged TWICE, each staging PACKED to
only the tiles its windows touch (aligned tiles 3p,3p+1 for even windows;
64-row-shifted tiles for odd windows), interleaved in consumption order
in ONE dram tensor (4 slots/pair, 44 slots, 5.8 MB total vs 4.3 MB for a
single full staging) — every window is then a 4-matmul aligned DFT with
the SAME weight pack.  88 fwd + 86 inv matmuls; PE busy ~39 us,
ACT ~41.5 and DVE ~41 are the co-limiting engines.

Freq packing (256-pt real DFT -> 256 real rows in 2 PSUM banks):
    X0 rows f=0..127:  Re X[f]
    X1 row 0: Re X[128] (Nyquist); rows 1..127: Im X[f]
Pointwise complex multiply, Y0 = A0*X0 + B0*X1 ; Y1 = A1*X1 + B1*X0, with
the omega residual folded into tap 0 host-side.

Engine law on TRN2 (measured): GpSimd tensor ops and DVE 2-port copy/cast
modes serialize on a shared SBUF port (exclusive lock), so GpSimd does NO
compute here (only SWDGE DMA issue) and all PSUM evacuation runs on
ScalarE.  Windows run in PAIRS but the pointwise stage is PER-WINDOW so
each window's DVE chain waits only on its own evacuation:
    ACT : per-window evac copy PSUM -> SBUF slice of x01 (FD=1024, ~1.0 us)
          + one fused SiLU over the pair's 3 finished output banks (1.5 us)
    DVE : per window: m0 = x*[A0|B0], m1 = x*[B1|A1] (FD=1024, 2x_1P,
          0.59 us each), y0/y1 bank adds (FD=512, 0.33 us each)
    PE  : fwd 4 (even) / 6 (odd) + inv 8 matmuls per pair (~213 ns each)
Inverse matmuls are emitted window-major; the pair's 3 output blocks live
in one 3-bank PSUM tile (split middle block accumulates both windows),
SiLU reads all 3 at once and one Pool-SWDGE DMA stores 384 output rows.

DMA choreography (the shared DMA-engine bus is granted roughly in request
order, so queue assignment = transfer priority): sync streams the packed
x (one 2-slot chunk so window 0 starts early, then 4-slot = 1-pair
chunks); scalar carries the 128 KB fwd-weight pack; Pool SWDGE (which
pipelines dispatches ~1.1 us apart instead of blocking per transfer)
carries the two coefficient-plane halves + inverse weights + output
stores.  The kco pack ships only its two m-planes (512 KB, halved vs the
pair-duplicated layout) since per-window muls read one plane slot.  The
last pair runs at single-window granularity with per-bank SiLU + store,
its PSUM banks borrowed from the fwd pool, so the drain is half-depth.
TimelineSim: 52.5 us (this session's baseline kernel: 62.3).
"""

import math
import numpy as np
from contextlib import ExitStack

import concourse.bass as bass
import concourse.tile as tile
from concourse import bacc, mybir
from concourse.bass_utils import run_bass_kernel_spmd

L, B, E, NDIM = 4096, 4, 1024, 16
N_CORES = 8
ESH = E // N_CORES            # 128 channels per core
F, T, C = 256, 32, 192        # DFT length, one-sided taps, hop
NW = (L + C - 1) // C         # 22 windows (last covers 64 outputs)
NP = (NW + 1) // 2            # 11 window pairs
FREE = B * ESH                # 512 free elements (b, chan)
NXT = 33                      # x tiles: rows [0, 4224), x at [T, T+L)
NBLK = L // 128               # 32 output blocks

F16 = mybir.dt.float16
F32 = mybir.dt.float32

LAST_RESULTS = None           # BassKernelResults of the most recent run
_CACHE: dict = {}


def _build_nc():
    nc = bacc.Bacc("TRN2", target_bir_lowering=False, debug=False,
                   num_devices=N_CORES)
    xs = nc.dram_tensor("xs", [128, 4 * NP, FREE], F16, kind="ExternalInput").ap()
    wfo = nc.dram_tensor("wfo", [128, 2, 2, 128], F16, kind="ExternalInput").ap()
    vi = nc.dram_tensor("vi", [128, 2, 4, 128], F16, kind="ExternalInput").ap()
    kco = nc.dram_tensor("kco", [128, 2, 2, FREE], F16,
                         kind="ExternalInput").ap()
    out = nc.dram_tensor("out", [128, NBLK, FREE], F16, kind="ExternalOutput").ap()

    with ExitStack() as ctx:
        tc = ctx.enter_context(tile.TileContext(nc))
        cpool = ctx.enter_context(tc.tile_pool(name="const", bufs=1))
        ppool = ctx.enter_context(tc.tile_pool(name="pw", bufs=2))
        opool = ctx.enter_context(tc.tile_pool(name="outp", bufs=2))
        ps_f = ctx.enter_context(tc.tile_pool(name="psf", bufs=1, space="PSUM"))
        ps_i = ctx.enter_context(tc.tile_pool(name="psi", bufs=1, space="PSUM"))

        # DMA queue split: sync = the packed x stream; scalar = fwd
        # weights; gpsimd/Pool = coefficient planes + inverse weights +
        # output stores (otherwise stores queue behind the x stream and
        # delay buffer recycling).
        x_all = cpool.tile([128, NXT, FREE], F16)
        nc.sync.dma_start(x_all[:, 0:4, :], xs[:, 0:4, :])
        # fwd weights in one small transfer; coefficient planes follow as
        # two halves on Pool (the m0 plane is all the first pointwise mul
        # needs, and it lands ~1.5us sooner than the full pack would).
        wfo_t = cpool.tile([128, 2, 2, 128], F16)
        nc.scalar.dma_start(wfo_t[:], wfo)

        # Pool's SWDGE pipelines dispatches (~1.1us apart) while the HWDGE
        # queues block per-transfer, so the coefficient halves + inverse
        # weights stream here without delaying the x stream or ACT's SEQ.
        k_t = cpool.tile([128, 2, 2, FREE], F16)
        vi_t = cpool.tile([128, 2, 4, 128], F16)
        nc.gpsimd.dma_start(k_t[:, 0, :, :], kco[:, 0, :, :])
        nc.gpsimd.dma_start(k_t[:, 1, :, :], kco[:, 1, :, :])
        nc.gpsimd.dma_start(vi_t[:], vi)
        for t0 in range(2, 4 * NP, 4):
            t1 = min(t0 + 4, 4 * NP)
            nc.sync.dma_start(x_all[:, t0:t1, :], xs[:, t0:t1, :])

        def fwd(w):
            """forward 256-pt real DFT of window w into its own 2-bank PSUM
            tile (double-buffered so the evacuation copy of window w runs
            while window w+1's matmuls stream).  Odd windows span 3 x tiles;
            edge chunks use zero-padded weight rows so every matmul stays
            full 128x128 config."""
            xw = ps_f.tile([128, 2, FREE], F32, tag="xw", name=f"xw_{w}",
                           bufs=2)
            a = 2 * w          # pair p slots: 4p,4p+1 aligned; 4p+2,4p+3 shifted
            for b in range(2):
                for k in range(2):
                    nc.tensor.matmul(xw[:, b, :], wfo_t[:, b, k, :],
                                     x_all[:, a + k, :],
                                     start=(k == 0), stop=(k == 1))
            return xw

        def pointwise(p, xh):
            """per-window elementwise: evacuate on ACT, multiply/add on DVE.
            Each window's mul/add chain depends only on its own evacuation,
            so window 2p's y-legs are ready one evac-copy earlier than a
            pair-fused chain would allow."""
            x01 = ppool.tile([128, 2, 2, FREE], F16, tag="x01", name=f"x01_{p}")
            y0 = ppool.tile([128, 2, FREE], F16, tag="y0", name=f"y0_{p}")
            y1 = ppool.tile([128, 2, FREE], F16, tag="y1", name=f"y1_{p}")
            for i, xw in enumerate(xh):
                nc.scalar.copy(x01[:, :, i, :], xw[:])
                for mp, yv in ((0, y0), (1, y1)):
                    m = ppool.tile([128, 2, FREE], F16, tag=f"m{mp}{i}",
                                   name=f"m{mp}_{p}_{i}")
                    nc.vector.tensor_mul(m[:], x01[:, :, i, :],
                                         k_t[:, mp, :, :])
                    nc.vector.tensor_add(yv[:, i, :], m[:, 0, :], m[:, 1, :])
            return y0, y1

        def inv_ranges(w):
            # (pair-slice, vseg, first_writer, window)
            if w % 2 == 0:
                return [(0, 0, True, w), (1, 1, True, w)]
            return [(1, 3, False, w), (2, 2, True, w)]

        def inv_window(p, yi, nb, w, y0, y1):
            """both legs of window w's inverse DFT, window-major so each
            window's matmuls wait only on its own y tiles"""
            ranges = [r for r in inv_ranges(w) if r[0] < nb]
            for sl, seg, first, _ in ranges:
                nc.tensor.matmul(yi[:, sl, :], vi_t[:, 0, seg, :],
                                 y0[:, w % 2, :], start=first, stop=False,
                                 skip_group_check=True)
            for sl, seg, first, _ in ranges:
                # slice 1 is shared by both windows; the odd window ends it
                last = sl != 1 or w % 2 == 1
                nc.tensor.matmul(yi[:, sl, :], vi_t[:, 1, seg, :],
                                 y1[:, w % 2, :], start=False, stop=last,
                                 skip_group_check=True)

        def silu_store(p, yi, nb):
            """fused SiLU over the pair's 3 banks + one 384-row store.
            Emitted one pair LATE so it sits behind the next pair's ACT
            evacuation copies instead of blocking them.  Stores ride the
            otherwise-idle Pool SWDGE queue; the final store uses sync
            (HWDGE, lower fixed latency) since sync has drained by then."""
            o_sb = opool.tile([128, 3, FREE], F16, tag=f"o{p % 2}", name=f"o{p}")
            nc.scalar.activation(o_sb[:, 0:nb, :], yi[:, 0:nb, :],
                                 mybir.ActivationFunctionType.Silu)
            eng = nc.sync if p == NP - 1 else nc.gpsimd
            eng.dma_start(out[:, 3 * p: 3 * p + nb, :], o_sb[:, 0:nb, :])

        # PE pre-warm: dummy matmuls keep the HAM activity monitor busy while
        # the first x tiles stream in, so real matmuls start near 2.4 GHz.
        # 8 is enough to cover the ~1.9us until wf lands without delaying
        # the first real DFT.
        warm = ps_i.tile([128, 3, FREE], F32, tag="yi", name="warm")
        garb = cpool.tile([128, 2, 128], F16)
        nc.vector.memset(garb[:], 0.25)
        for r in range(8):
            nc.tensor.matmul(warm[:, 0, 0:256], garb[:, 0, :],
                             garb[:, :, :], start=(r == 0), stop=(r == 7))
        # preload the sigmoid/silu ACT table set (~2.7 us) during the ramp so
        # the first real SiLU doesn't stall the ACT queue mid-pipeline.
        o_warm = opool.tile([128, 3, FREE], F16, tag="o0", name="o_warm")
        nc.scalar.activation(o_warm[:, 0, 0:8], warm[:, 0, 0:8],
                             mybir.ActivationFunctionType.Silu)

        # pipeline: evacuate pair p (ACT), queue fwd(p+1) (waits only on the
        # evacuation), then pair p's DVE pointwise + inverse.
        xh_cur = [fwd(0), fwd(1)]
        pend = None
        for p in range(NP - 1):
            y0, y1 = pointwise(p, xh_cur)
            if pend is not None:
                silu_store(p - 1, *pend)
            xh_next = [fwd(2 * p + 2)]
            if 2 * p + 3 < NW:
                xh_next.append(fwd(2 * p + 3))
            nb = min(3, NBLK - 3 * p)
            yi = ps_i.tile([128, 3, FREE], F32, tag="yi", name=f"yi{p}")
            inv_window(p, yi, nb, 2 * p, y0, y1)
            inv_window(p, yi, nb, 2 * p + 1, y0, y1)
            pend = (yi, nb)
            xh_cur = xh_next

        # Last pair (windows NW-2, NW-1; output blocks NBLK-2, NBLK-1) runs
        # at single-window granularity with per-bank SiLU + store so the
        # pipeline drains at half depth: block NBLK-2 completes and stores
        # from window NW-2 alone while window NW-1 is still in pointwise.
        p = NP - 1
        # the tail's 2 output banks live in the forward-DFT PSUM pool (its
        # buffers free once the last windows are evacuated) so the inverse
        # legs do NOT wait for silu(p-1) to drain the shared yi buffer.
        yi = ps_f.tile([128, 2, FREE], F32, tag="xw", name=f"yi{p}", bufs=2)
        x01 = ppool.tile([128, 2, 2, FREE], F16, tag="x01", name=f"x01_{p}")
        o_sb = opool.tile([128, 3, FREE], F16, tag=f"o{p % 2}", name=f"o{p}")

        def tail_window(i):
            """pointwise + inverse legs for window 2p+i at FD half-size"""
            ym = []
            for mp in range(2):
                m = ppool.tile([128, 2, FREE], F16, tag=f"m{mp}{i}",
                               name=f"mL{i}_{mp}")
                nc.vector.tensor_mul(m[:], x01[:, :, i, :], k_t[:, mp, :, :])
                yv = ppool.tile([128, FREE], F16, tag=f"y{mp}",
                                name=f"yL{i}_{mp}")
                nc.vector.tensor_add(yv[:], m[:, 0, :], m[:, 1, :])
                ym.append(yv)
            # even window fills bank0 + starts bank1; odd window finishes bank1
            segs = [(0, 0), (1, 1)] if i == 0 else [(1, 3)]
            for leg in range(2):
                for sl, seg in segs:
                    nc.tensor.matmul(yi[:, sl, :], vi_t[:, leg, seg, :],
                                     ym[leg][:],
                                     start=(leg == 0 and i == 0),
                                     stop=(leg == 1 and (sl == 0 or i == 1)),
                                     skip_group_check=True)

        nc.scalar.copy(x01[:, :, 0, :], xh_cur[0][:])
        nc.scalar.copy(x01[:, :, 1, :], xh_cur[1][:])
        tail_window(0)
        nc.scalar.activation(o_sb[:, 0, :], yi[:, 0, :],
                             mybir.ActivationFunctionType.Silu)
        nc.gpsimd.dma_start(out[:, 3 * p, :], o_sb[:, 0, :])
        silu_store(p - 1, *pend)
        tail_window(1)
        nc.scalar.activation(o_sb[:, 1, :], yi[:, 1, :],
                             mybir.ActivationFunctionType.Silu)
        nc.sync.dma_start(out[:, 3 * p + 1, :], o_sb[:, 1, :])
    nc.compile()
    return nc


def _host_prep(x, alpha, delta, beta, gamma, omega):
    """Fold EMA params into freq-domain coefficient planes + DFT matrices."""
    a = 1.0 / (1.0 + np.exp(-alpha.astype(np.float64)))
    d = 1.0 / (1.0 + np.exp(-delta.astype(np.float64)))
    q = 1.0 - a * d                               # (2E, 16, 1)
    w = (a * beta.astype(np.float64))[:, :, 0] * gamma.astype(np.float64)
    w *= math.sqrt(1.0 / NDIM)                    # (2E, 16)
    tau = np.arange(128)
    kern = (w[:, :, None] * q[:, :, 0:1] ** tau[None, None, :]).sum(1)  # (2E,128)
    k1, k2 = kern[:E], kern[E:]
    kc = np.zeros((E, F))
    kc[:, 0:128] = k1
    kc[:, F - 127:] = k2[:, :127][:, ::-1]        # slot F-i holds k2[i-1]
    kc[:, 0] += omega.astype(np.float64)          # residual == omega on tap 0
    Khat = np.fft.rfft(kc, axis=1)                # (E, 129)
    KRe, KIm = Khat.real, Khat.imag

    planes = np.zeros((4, 128, E))                # A0, B0, A1, B1
    planes[0] = KRe[:, 0:128].T
    planes[1, 1:] = -KIm[:, 1:128].T
    planes[2, 0] = KRe[:, 128]
    planes[2, 1:] = KRe[:, 1:128].T
    planes[3, 1:] = KIm[:, 1:128].T

    # forward DFT lhsT packs: W0 (Re rows), W1 (Nyquist + Im rows) [256, 128]
    t_ = np.arange(F)
    fr = np.arange(128)
    W0 = np.cos(2 * np.pi * np.outer(t_, fr) / F)
    W1 = np.empty((F, 128))
    W1[:, 0] = np.cos(np.pi * t_)
    W1[:, 1:] = -np.sin(2 * np.pi * np.outer(t_, fr[1:]) / F)
    Wb = np.stack([W0, W1], axis=0)               # (2, 256, 128)
    # both window parities use the same aligned chunks (odd windows read
    # the 64-row-shifted x staging)
    wfo = np.zeros((128, 2, 2, 128))
    for k in range(2):
        wfo[:, :, k, :] = Wb[:, 128 * k:128 * (k + 1), :].transpose(1, 0, 2)

    # inverse lhsT: V0/V1 [128 freq, 192 outs], zero-padded per range
    jj = np.arange(C) + T
    c_f = np.where(fr == 0, 1.0, 2.0)
    V0 = c_f[:, None] * np.cos(2 * np.pi * np.outer(fr, jj) / F) / F
    V1 = np.empty((128, C))
    V1[0] = ((-1.0) ** jj) / F
    V1[1:] = -2 * np.sin(2 * np.pi * np.outer(fr[1:], jj) / F) / F
    Vb = np.stack([V0, V1], axis=0)               # (2, 128, 192)
    vi = np.zeros((128, 2, 4, 128))
    vi[:, :, 0, :] = Vb[:, :, 0:128].transpose(1, 0, 2)      # even range A
    vi[:, :, 1, 0:64] = Vb[:, :, 128:192].transpose(1, 0, 2)  # even range B
    vi[:, :, 2, :] = Vb[:, :, 64:192].transpose(1, 0, 2)     # odd range B
    vi[:, :, 3, 64:128] = Vb[:, :, 0:64].transpose(1, 0, 2)  # odd range A

    xbase = np.zeros((NXT * 128 + 64, B, E), np.float16)
    xbase[T:T + L] = x.astype(np.float16)
    # partition-major staging: [128, NXT, B, E] so DMA partition lines are
    # long and contiguous (strided 1KB lines measured ~5x slower).
    xpad = np.ascontiguousarray(
        xbase[:NXT * 128].reshape(NXT, 128, B, E).transpose(1, 0, 2, 3))
    # shifted staging, packed: odd window w = 2j+1 contracts xpad2 tiles
    # (3j+1, 3j+2) where xpad2 rows = xbase rows shifted down 64
    xpad2 = np.ascontiguousarray(
        xbase[64:64 + NXT * 128].reshape(NXT, 128, B, E).transpose(1, 0, 2, 3))
    # interleave in consumption order: pair p = [aligned 3p, 3p+1,
    # shifted 3p+1, 3p+2]  (shifted tile t = xbase rows 64+128t..192+128t)
    xin = np.empty((128, 4 * NP, B, E), np.float16)
    for p in range(NP):
        xin[:, 4 * p + 0] = xpad[:, 3 * p + 0]
        xin[:, 4 * p + 1] = xpad[:, 3 * p + 1]
        xin[:, 4 * p + 2] = xpad2[:, 3 * p + 1]
        xin[:, 4 * p + 3] = xpad2[:, 3 * p + 2]

    wfo16 = np.ascontiguousarray(wfo.astype(np.float16))
    vi16 = np.ascontiguousarray(vi.astype(np.float16))
    # fused-mul plane packs: m0 halves [A0 | B0], m1 halves [B1 | A1];
    # the window-pair duplication happens on-chip (one DVE copy).
    pk = np.stack([np.stack([planes[0], planes[1]]),
                   np.stack([planes[3], planes[2]])])   # (2, 2, 128, E)
    in_maps = []
    for core in range(N_CORES):
        sl = slice(core * ESH, (core + 1) * ESH)
        kc1 = np.broadcast_to(
            pk.reshape(2, 2, 128, 1, E)[:, :, :, :, sl],
            (2, 2, 128, B, ESH)).reshape(2, 2, 128, FREE)
        kc1 = kc1.transpose(2, 0, 1, 3)           # (128, m, half, FREE)
        in_maps.append({
            "xs": np.ascontiguousarray(xin[:, :, :, sl]).reshape(128, 4 * NP, FREE),
            "wfo": wfo16,
            "vi": vi16,
            "kco": np.ascontiguousarray(kc1.astype(np.float16)),
        })
    return in_maps


def kernel(x, alpha, delta, beta, gamma, omega):
    global LAST_RESULTS
    if "nc" not in _CACHE:
        _CACHE["nc"] = _build_nc()
    nc = _CACHE["nc"]
    in_maps = _host_prep(x, alpha, delta, beta, gamma, omega)
    res = run_bass_kernel_spmd(nc, in_maps, core_ids=list(range(N_CORES)))
    LAST_RESULTS = res
    outs = []
    for c in range(N_CORES):
        o = res.results[c]["out"].reshape(128, NBLK, B, ESH)
        outs.append(o.transpose(1, 0, 2, 3).reshape(L, B, ESH))
    return np.concatenate(outs, axis=2).astype(np.float32)



# revision 71
# speedup vs baseline: 1.0067x; 1.0067x over previous
"""MultiHeadEMA (MEGA bidirectional EMA + residual + SiLU) on 8 Trainium2 cores.

Strategy
--------
Per channel d (E=1024, B=4, L=4096):
    y[n] = silu( sum_{m<=n} x[m] k1[d, n-m] + sum_{m>n} x[m] k2[d, m-n-1]
                 + omega[d] x[n] )
with k1/k2 16-term geometric mixtures, q = 1 - sigmoid(a)*sigmoid(d) <= 0.865.
q^32 tail: worst-channel L1 1.4e-2 << 2e-2 * scale(16.9), so the length-2L FFT
conv reduces to a +-T=32-tap banded conv done by overlap-save with DFT F=256,
hop C=192 (22 windows).  E sharded 8 ways (128 ch/core, FREE = B*128 = 512).

Matmul cost on TRN2 is free-dim cycles (~216 ns at N=512 fp16), so every
matmul is full 128x128 config.  x is staged TWICE, each staging PACKED to
only the tiles its windows touch (aligned tiles 3p,3p+1 for even windows;
64-row-shifted tiles for odd windows), interleaved in consumption order
in ONE dram tensor (4 slots/pair, 44 slots, 5.8 MB total vs 4.3 MB for a
single full staging) — every window is then a 4-matmul aligned DFT with
the SAME weight pack.  88 fwd + 86 inv matmuls; PE busy ~39 us,
ACT ~41.5 and DVE ~41 are the co-limiting engines.

Freq packing (256-pt real DFT -> 256 real rows in 2 PSUM banks):
    X0 rows f=0..127:  Re X[f]
    X1 row 0: Re X[128] (Nyquist); rows 1..127: Im X[f]
Pointwise complex multiply, Y0 = A0*X0 + B0*X1 ; Y1 = A1*X1 + B1*X0, with
the omega residual folded into tap 0 host-side.

Engine law on TRN2 (measured): GpSimd tensor ops and DVE 2-port copy/cast
modes serialize on a shared SBUF port (exclusive lock), so GpSimd does NO
compute here (only SWDGE DMA issue) and all PSUM evacuation runs on
ScalarE.  Windows run in PAIRS but the pointwise stage is PER-WINDOW so
each window's DVE chain waits only on its own evacuation:
    ACT : per-window evac copy PSUM -> SBUF slice of x01 (FD=1024, ~1.0 us)
          + one fused SiLU over the pair's 3 finished output banks (1.5 us)
    DVE : per window: m0 = x*[A0|B0], m1 = x*[B1|A1] (FD=1024, 2x_1P,
          0.59 us each), y0/y1 bank adds (FD=512, 0.33 us each)
    PE  : fwd 4 (even) / 6 (odd) + inv 8 matmuls per pair (~213 ns each)
Inverse matmuls are emitted window-major; the pair's 3 output blocks live
in one 3-bank PSUM tile (split middle block accumulates both windows),
SiLU reads all 3 at once and one Pool-SWDGE DMA stores 384 output rows.

DMA choreography (the shared DMA-engine bus is granted roughly in request
order, so queue assignment = transfer priority): sync streams the packed
x in 2-slot (= 1-window) chunks so each window's data lands just ahead
of its DFT; scalar carries the 128 KB fwd-weight pack; Pool SWDGE (which
pipelines dispatches ~1.1 us apart instead of blocking per transfer)
carries the two coefficient-plane halves + inverse weights + output
stores.  The kco pack ships only its two m-planes (512 KB, halved vs the
pair-duplicated layout) since per-window muls read one plane slot.  The
last pair runs at single-window granularity with per-bank SiLU + store,
its PSUM banks borrowed from the fwd pool, so the drain is half-depth.
TimelineSim: 52.2 us (this session's baseline kernel: 62.3).
"""

import math
import numpy as np
from contextlib import ExitStack

import concourse.bass as bass
import concourse.tile as tile
from concourse import bacc, mybir
from concourse.bass_utils import run_bass_kernel_spmd

L, B, E, NDIM = 4096, 4, 1024, 16
N_CORES = 8
ESH = E // N_CORES            # 128 channels per core
F, T, C = 256, 32, 192        # DFT length, one-sided taps, hop
NW = (L + C - 1) // C         # 22 windows (last covers 64 outputs)
NP = (NW + 1) // 2            # 11 window pairs
FREE = B * ESH                # 512 free elements (b, chan)
NXT = 33                      # x tiles: rows [0, 4224), x at [T, T+L)
NBLK = L // 128               # 32 output blocks

F16 = mybir.dt.float16
F32 = mybir.dt.float32

LAST_RESULTS = None           # BassKernelResults of the most recent run
_CACHE: dict = {}


def _build_nc():
    nc = bacc.Bacc("TRN2", target_bir_lowering=False, debug=False,
                   num_devices=N_CORES)
    xs = nc.dram_tensor("xs", [128, 4 * NP, FREE], F16, kind="ExternalInput").ap()
    wfo = nc.dram_tensor("wfo", [128, 2, 2, 128], F16, kind="ExternalInput").ap()
    vi = nc.dram_tensor("vi", [128, 2, 4, 128], F16, kind="ExternalInput").ap()
    kco = nc.dram_tensor("kco", [128, 2, 2, FREE], F16,
                         kind="ExternalInput").ap()
    out = nc.dram_tensor("out", [128, NBLK, FREE], F16, kind="ExternalOutput").ap()

    with ExitStack() as ctx:
        tc = ctx.enter_context(tile.TileContext(nc))
        cpool = ctx.enter_context(tc.tile_pool(name="const", bufs=1))
        ppool = ctx.enter_context(tc.tile_pool(name="pw", bufs=2))
        opool = ctx.enter_context(tc.tile_pool(name="outp", bufs=2))
        ps_f = ctx.enter_context(tc.tile_pool(name="psf", bufs=1, space="PSUM"))
        ps_i = ctx.enter_context(tc.tile_pool(name="psi", bufs=1, space="PSUM"))

        # DMA queue split: sync = the packed x stream; scalar = fwd
        # weights; gpsimd/Pool = coefficient planes + inverse weights +
        # output stores (otherwise stores queue behind the x stream and
        # delay buffer recycling).
        x_all = cpool.tile([128, NXT, FREE], F16)
        nc.sync.dma_start(x_all[:, 0:4, :], xs[:, 0:4, :])
        # fwd weights in one small transfer; coefficient planes follow as
        # two halves on Pool (the m0 plane is all the first pointwise mul
        # needs, and it lands ~1.5us sooner than the full pack would).
        wfo_t = cpool.tile([128, 2, 2, 128], F16)
        nc.scalar.dma_start(wfo_t[:], wfo)

        # Pool's SWDGE pipelines dispatches (~1.1us apart) while the HWDGE
        # queues block per-transfer, so the coefficient halves + inverse
        # weights stream here without delaying the x stream or ACT's SEQ.
        k_t = cpool.tile([128, 2, 2, FREE], F16)
        vi_t = cpool.tile([128, 2, 4, 128], F16)
        nc.gpsimd.dma_start(k_t[:, 0, :, :], kco[:, 0, :, :])
        nc.gpsimd.dma_start(k_t[:, 1, :, :], kco[:, 1, :, :])
        nc.gpsimd.dma_start(vi_t[:], vi)
        for t0 in range(2, 4 * NP, 2):
            t1 = min(t0 + 2, 4 * NP)
            nc.sync.dma_start(x_all[:, t0:t1, :], xs[:, t0:t1, :])

        def fwd(w):
            """forward 256-pt real DFT of window w into its own 2-bank PSUM
            tile (double-buffered so the evacuation copy of window w runs
            while window w+1's matmuls stream).  Both parities contract 2
            aligned slots of the packed staging with the same weights."""
            xw = ps_f.tile([128, 2, FREE], F32, tag="xw", name=f"xw_{w}",
                           bufs=2)
            a = 2 * w          # pair p slots: 4p,4p+1 aligned; 4p+2,4p+3 shifted
            for b in range(2):
                for k in range(2):
                    nc.tensor.matmul(xw[:, b, :], wfo_t[:, b, k, :],
                                     x_all[:, a + k, :],
                                     start=(k == 0), stop=(k == 1))
            return xw

        def pointwise(p, xh):
            """per-window elementwise: evacuate on ACT, multiply/add on DVE.
            Each window's mul/add chain depends only on its own evacuation,
            so window 2p's y-legs are ready one evac-copy earlier than a
            pair-fused chain would allow."""
            x01 = ppool.tile([128, 2, 2, FREE], F16, tag="x01", name=f"x01_{p}")
            y0 = ppool.tile([128, 2, FREE], F16, tag="y0", name=f"y0_{p}")
            y1 = ppool.tile([128, 2, FREE], F16, tag="y1", name=f"y1_{p}")
            for i, xw in enumerate(xh):
                nc.scalar.copy(x01[:, :, i, :], xw[:])
                for mp, yv in ((0, y0), (1, y1)):
                    m = ppool.tile([128, 2, FREE], F16, tag=f"m{mp}{i}",
                                   name=f"m{mp}_{p}_{i}")
                    nc.vector.tensor_mul(m[:], x01[:, :, i, :],
                                         k_t[:, mp, :, :])
                    nc.vector.tensor_add(yv[:, i, :], m[:, 0, :], m[:, 1, :])
            return y0, y1

        def inv_ranges(w):
            # (pair-slice, vseg, first_writer, window)
            if w % 2 == 0:
                return [(0, 0, True, w), (1, 1, True, w)]
            return [(1, 3, False, w), (2, 2, True, w)]

        def inv_window(p, yi, nb, w, y0, y1):
            """both legs of window w's inverse DFT, window-major so each
            window's matmuls wait only on its own y tiles"""
            ranges = [r for r in inv_ranges(w) if r[0] < nb]
            for sl, seg, first, _ in ranges:
                nc.tensor.matmul(yi[:, sl, :], vi_t[:, 0, seg, :],
                                 y0[:, w % 2, :], start=first, stop=False,
                                 skip_group_check=True)
            for sl, seg, first, _ in ranges:
                # slice 1 is shared by both windows; the odd window ends it
                last = sl != 1 or w % 2 == 1
                nc.tensor.matmul(yi[:, sl, :], vi_t[:, 1, seg, :],
                                 y1[:, w % 2, :], start=False, stop=last,
                                 skip_group_check=True)

        def silu_store(p, yi, nb):
            """fused SiLU over the pair's 3 banks + one 384-row store.
            Emitted one pair LATE so it sits behind the next pair's ACT
            evacuation copies instead of blocking them.  Stores ride the
            otherwise-idle Pool SWDGE queue; the final store uses sync
            (HWDGE, lower fixed latency) since sync has drained by then."""
            o_sb = opool.tile([128, 3, FREE], F16, tag=f"o{p % 2}", name=f"o{p}")
            nc.scalar.activation(o_sb[:, 0:nb, :], yi[:, 0:nb, :],
                                 mybir.ActivationFunctionType.Silu)
            eng = nc.sync if p == NP - 1 else nc.gpsimd
            eng.dma_start(out[:, 3 * p: 3 * p + nb, :], o_sb[:, 0:nb, :])

        # PE pre-warm: dummy matmuls keep the HAM activity monitor busy while
        # the first x tiles stream in, so real matmuls start near 2.4 GHz.
        # 8 is enough to cover the ~1.9us until wf lands without delaying
        # the first real DFT.
        warm = ps_i.tile([128, 3, FREE], F32, tag="yi", name="warm")
        garb = cpool.tile([128, 2, 128], F16)
        nc.vector.memset(garb[:], 0.25)
        for r in range(8):
            nc.tensor.matmul(warm[:, 0, 0:256], garb[:, 0, :],
                             garb[:, :, :], start=(r == 0), stop=(r == 7))
        # preload the sigmoid/silu ACT table set (~2.7 us) during the ramp so
        # the first real SiLU doesn't stall the ACT queue mid-pipeline.
        o_warm = opool.tile([128, 3, FREE], F16, tag="o0", name="o_warm")
        nc.scalar.activation(o_warm[:, 0, 0:8], warm[:, 0, 0:8],
                             mybir.ActivationFunctionType.Silu)

        # pipeline: evacuate pair p (ACT), queue fwd(p+1) (waits only on the
        # evacuation), then pair p's DVE pointwise + inverse.
        xh_cur = [fwd(0), fwd(1)]
        pend = None
        for p in range(NP - 1):
            y0, y1 = pointwise(p, xh_cur)
            if pend is not None:
                silu_store(p - 1, *pend)
            xh_next = [fwd(2 * p + 2)]
            if 2 * p + 3 < NW:
                xh_next.append(fwd(2 * p + 3))
            nb = min(3, NBLK - 3 * p)
            yi = ps_i.tile([128, 3, FREE], F32, tag="yi", name=f"yi{p}")
            inv_window(p, yi, nb, 2 * p, y0, y1)
            inv_window(p, yi, nb, 2 * p + 1, y0, y1)
            pend = (yi, nb)
            xh_cur = xh_next

        # Last pair (windows NW-2, NW-1; output blocks NBLK-2, NBLK-1) runs
        # at single-window granularity with per-bank SiLU + store so the
        # pipeline drains at half depth: block NBLK-2 completes and stores
        # from window NW-2 alone while window NW-1 is still in pointwise.
        p = NP - 1
        # the tail's 2 output banks live in the forward-DFT PSUM pool (its
        # buffers free once the last windows are evacuated) so the inverse
        # legs do NOT wait for silu(p-1) to drain the shared yi buffer.
        yi = ps_f.tile([128, 2, FREE], F32, tag="xw", name=f"yi{p}", bufs=2)
        x01 = ppool.tile([128, 2, 2, FREE], F16, tag="x01", name=f"x01_{p}")
        o_sb = opool.tile([128, 3, FREE], F16, tag=f"o{p % 2}", name=f"o{p}")

        def tail_window(i):
            """pointwise + inverse legs for window 2p+i at FD half-size"""
            ym = []
            for mp in range(2):
                m = ppool.tile([128, 2, FREE], F16, tag=f"m{mp}{i}",
                               name=f"mL{i}_{mp}")
                nc.vector.tensor_mul(m[:], x01[:, :, i, :], k_t[:, mp, :, :])
                yv = ppool.tile([128, FREE], F16, tag=f"y{mp}",
                                name=f"yL{i}_{mp}")
                nc.vector.tensor_add(yv[:], m[:, 0, :], m[:, 1, :])
                ym.append(yv)
            # even window fills bank0 + starts bank1; odd window finishes bank1
            segs = [(0, 0), (1, 1)] if i == 0 else [(1, 3)]
            for leg in range(2):
                for sl, seg in segs:
                    nc.tensor.matmul(yi[:, sl, :], vi_t[:, leg, seg, :],
                                     ym[leg][:],
                                     start=(leg == 0 and i == 0),
                                     stop=(leg == 1 and (sl == 0 or i == 1)),
                                     skip_group_check=True)

        nc.scalar.copy(x01[:, :, 0, :], xh_cur[0][:])
        nc.scalar.copy(x01[:, :, 1, :], xh_cur[1][:])
        tail_window(0)
        nc.scalar.activation(o_sb[:, 0, :], yi[:, 0, :],
                             mybir.ActivationFunctionType.Silu)
        nc.gpsimd.dma_start(out[:, 3 * p, :], o_sb[:, 0, :])
        silu_store(p - 1, *pend)
        tail_window(1)
        nc.scalar.activation(o_sb[:, 1, :], yi[:, 1, :],
                             mybir.ActivationFunctionType.Silu)
        nc.sync.dma_start(out[:, 3 * p + 1, :], o_sb[:, 1, :])
    nc.compile()
    return nc


def _host_prep(x, alpha, delta, beta, gamma, omega):
    """Fold EMA params into freq-domain coefficient planes + DFT matrices."""
    a = 1.0 / (1.0 + np.exp(-alpha.astype(np.float64)))
    d = 1.0 / (1.0 + np.exp(-delta.astype(np.float64)))
    q = 1.0 - a * d                               # (2E, 16, 1)
    w = (a * beta.astype(np.float64))[:, :, 0] * gamma.astype(np.float64)
    w *= math.sqrt(1.0 / NDIM)                    # (2E, 16)
    tau = np.arange(128)
    kern = (w[:, :, None] * q[:, :, 0:1] ** tau[None, None, :]).sum(1)  # (2E,128)
    k1, k2 = kern[:E], kern[E:]
    kc = np.zeros((E, F))
    kc[:, 0:128] = k1
    kc[:, F - 127:] = k2[:, :127][:, ::-1]        # slot F-i holds k2[i-1]
    kc[:, 0] += omega.astype(np.float64)          # residual == omega on tap 0
    Khat = np.fft.rfft(kc, axis=1)                # (E, 129)
    KRe, KIm = Khat.real, Khat.imag

    planes = np.zeros((4, 128, E))                # A0, B0, A1, B1
    planes[0] = KRe[:, 0:128].T
    planes[1, 1:] = -KIm[:, 1:128].T
    planes[2, 0] = KRe[:, 128]
    planes[2, 1:] = KRe[:, 1:128].T
    planes[3, 1:] = KIm[:, 1:128].T

    # forward DFT lhsT packs: W0 (Re rows), W1 (Nyquist + Im rows) [256, 128]
    t_ = np.arange(F)
    fr = np.arange(128)
    W0 = np.cos(2 * np.pi * np.outer(t_, fr) / F)
    W1 = np.empty((F, 128))
    W1[:, 0] = np.cos(np.pi * t_)
    W1[:, 1:] = -np.sin(2 * np.pi * np.outer(t_, fr[1:]) / F)
    Wb = np.stack([W0, W1], axis=0)               # (2, 256, 128)
    # both window parities use the same aligned chunks (odd windows read
    # the 64-row-shifted x staging)
    wfo = np.zeros((128, 2, 2, 128))
    for k in range(2):
        wfo[:, :, k, :] = Wb[:, 128 * k:128 * (k + 1), :].transpose(1, 0, 2)

    # inverse lhsT: V0/V1 [128 freq, 192 outs], zero-padded per range
    jj = np.arange(C) + T
    c_f = np.where(fr == 0, 1.0, 2.0)
    V0 = c_f[:, None] * np.cos(2 * np.pi * np.outer(fr, jj) / F) / F
    V1 = np.empty((128, C))
    V1[0] = ((-1.0) ** jj) / F
    V1[1:] = -2 * np.sin(2 * np.pi * np.outer(fr[1:], jj) / F) / F
    Vb = np.stack([V0, V1], axis=0)               # (2, 128, 192)
    vi = np.zeros((128, 2, 4, 128))
    vi[:, :, 0, :] = Vb[:, :, 0:128].transpose(1, 0, 2)      # even range A
    vi[:, :, 1, 0:64] = Vb[:, :, 128:192].transpose(1, 0, 2)  # even range B
    vi[:, :, 2, :] = Vb[:, :, 64:192].transpose(1, 0, 2)     # odd range B
    vi[:, :, 3, 64:128] = Vb[:, :, 0:64].transpose(1, 0, 2)  # odd range A

    xbase = np.zeros((NXT * 128 + 64, B, E), np.float16)
    xbase[T:T + L] = x.astype(np.float16)
    # partition-major staging: [128, NXT, B, E] so DMA partition lines are
    # long and contiguous (strided 1KB lines measured ~5x slower).
    xpad = np.ascontiguousarray(
        xbase[:NXT * 128].reshape(NXT, 128, B, E).transpose(1, 0, 2, 3))
    # shifted staging, packed: odd window w = 2j+1 contracts xpad2 tiles
    # (3j+1, 3j+2) where xpad2 rows = xbase rows shifted down 64
    xpad2 = np.ascontiguousarray(
        xbase[64:64 + NXT * 128].reshape(NXT, 128, B, E).transpose(1, 0, 2, 3))
    # interleave in consumption order: pair p = [aligned 3p, 3p+1,
    # shifted 3p+1, 3p+2]  (shifted tile t = xbase rows 64+128t..192+128t)
    xin = np.empty((128, 4 * NP, B, E), np.float16)
    for p in range(NP):
        xin[:, 4 * p + 0] = xpad[:, 3 * p + 0]
        xin[:, 4 * p + 1] = xpad[:, 3 * p + 1]
        xin[:, 4 * p + 2] = xpad2[:, 3 * p + 1]
        xin[:, 4 * p + 3] = xpad2[:, 3 * p + 2]

    wfo16 = np.ascontiguousarray(wfo.astype(np.float16))
    vi16 = np.ascontiguousarray(vi.astype(np.float16))
    # fused-mul plane packs: m0 halves [A0 | B0], m1 halves [B1 | A1];
    # the window-pair duplication happens on-chip (one DVE copy).
    pk = np.stack([np.stack([planes[0], planes[1]]),
                   np.stack([planes[3], planes[2]])])   # (2, 2, 128, E)
    in_maps = []
    for core in range(N_CORES):
        sl = slice(core * ESH, (core + 1) * ESH)
        kc1 = np.broadcast_to(
            pk.reshape(2, 2, 128, 1, E)[:, :, :, :, sl],
            (2, 2, 128, B, ESH)).reshape(2, 2, 128, FREE)
        kc1 = kc1.transpose(2, 0, 1, 3)           # (128, m, half, FREE)
        in_maps.append({
            "xs": np.ascontiguousarray(xin[:, :, :, sl]).reshape(128, 4 * NP, FREE),
            "wfo": wfo16,
            "vi": vi16,
            "kco": np.ascontiguousarray(kc1.astype(np.float16)),
        })
    return in_maps


def kernel(x, alpha, delta, beta, gamma, omega):
    global LAST_RESULTS
    if "nc" not in _CACHE:
        _CACHE["nc"] = _build_nc()
    nc = _CACHE["nc"]
    in_maps = _host_prep(x, alpha, delta, beta, gamma, omega)
    res = run_bass_kernel_spmd(nc, in_maps, core_ids=list(range(N_CORES)))
    LAST_RESULTS = res
    outs = []
    for c in range(N_CORES):
        o = res.results[c]["out"].reshape(128, NBLK, B, ESH)
        outs.append(o.transpose(1, 0, 2, 3).reshape(L, B, ESH))
    return np.concatenate(outs, axis=2).astype(np.float32)

